# revision 18
# baseline (speedup 1.0000x reference)
"""MemAttention Trainium2 kernel (8 NeuronCores, SPMD).

Math (see reference):
  q = gelu(query @ Wq.T + bq); k = gelu(key @ Wk.T + bk)        (erf gelu)
  mem = lam*memory + (1-lam)*q                                  (L == S == MAXL here)
  per (batch n, head h):  out = tril(qh @ kh.T) @ memh          (no softmax)
  out = LayerNorm_E(out) * ln_w + ln_b

Sharding: tensor-parallel over heads. Core c owns heads {2c, 2c+1} == E-slice
[128c, 128c+128). Each core reads the full (host-pre-transposed, bf16) query/key,
projects onto its 128-wide weight slice producing qT/kT in [head*d, token]
layout, and runs attention for its heads over all 4 batches.

Attention uses the chunked linear-attention form (exact reassociation of the
causal masked product):
  A_i = sum_{s < i*C} k[s] (x) mem[s]          (d x d running state)
  out[chunk i] = tril(q_i k_i^T) @ mem_i + q_i @ A_i
  A_{i+1} = A_i + k_i^T @ mem_i
which needs O(L*C) score work instead of O(L^2).

LayerNorm is over the full E=1024, which is sharded across cores: each core
contributes per-row (mean/8, E[x^2]/8) of its 128 columns; an 8-core
AllReduce(add) per batch yields global stats; each core then normalizes and
writes its own (L, N, 128) output slice; the host concatenates slices.

Scheduling notes (engines execute their static order in-order, so emission
order is the schedule):
 - projection row-tiles and the attention chunks they feed are interleaved,
   keeping PE busy while the big input DMA streams;
 - the per-batch stats AllReduce is kicked off right after batch n's chunks,
   but the LN application for batch n is emitted one batch later so no engine
   sits waiting on the collective;
 - LN scale/shift runs on the (otherwise idle) GPSIMD engine.
"""

import numpy as np
import ml_dtypes

import concourse.bass as bass
import concourse.mybir as mybir
import concourse.tile as tile
from concourse.bass_utils import run_bass_kernel_spmd
from concourse.masks import make_identity, make_upper_triangular

# ---------------------------------------------------------------------------
# Workaround: the walrus build in this container accepts only one sync-wait
# per instruction, but the Tile scheduler emits multi-wait Drains. Hoist the
# extra waits onto inserted NoOps (same engine, so execution order preserves
# semantics). Patched into both the native and the axon/PJRT compile paths.
# ---------------------------------------------------------------------------
import orjson

_MAX_WAITS = 1
_patch_done = False


def _split_waits(bir_json: bytes) -> bytes:
    d = orjson.loads(bir_json)
    n = 0
    for f in d.get("functions", []):
        for bb in f.get("blocks", []):
            instructions = bb.get("instructions")
            if not instructions:
                continue
            out = []
            changed = False
            for ins in instructions:
                si = ins.get("sync_info")
                waits = (si or {}).get("on_wait") or []
                if len(waits) > _MAX_WAITS:
                    changed = True
                    extra, keep = waits[:-_MAX_WAITS], waits[-_MAX_WAITS:]
                    for w in extra:
                        n += 1
                        out.append(
                            {
                                "debug": ins.get("debug", 0),
                                "engine": ins["engine"],
                                "ins": [],
                                "name": f"{ins.get('name', 'I')}-ws{n}",
                                "opcode": "NoOp",
                                "outs": [],
                                "sync_info": {"on_update": [], "on_wait": [w]},
                            }
                        )
                    si["on_wait"] = keep
                out.append(ins)
            if changed:
                bb["instructions"] = out
    return orjson.dumps(d)


def _install_patch():
    global _patch_done
    if _patch_done:
        return
    _patch_done = True
    import concourse.bass_utils as bass_utils
    import concourse.bass2jax as bass2jax

    orig = bass_utils.compile_bir_kernel

    def patched(bir_json, tmpdir, neff_name="file.neff"):
        return orig(_split_waits(bir_json), tmpdir, neff_name)

    bass_utils.compile_bir_kernel = patched
    bass2jax.compile_bir_kernel = patched


# ---------------------------------------------------------------------------
# Problem constants (hardcoded per contest contract)
# ---------------------------------------------------------------------------
L = 2048          # query length (== S == MAXL)
N = 4             # batch
E = 1024          # embed dim
H = 16            # heads
D = E // H        # head dim, 64
LAM = 0.001
LN_EPS = 1e-5
NCORES = 8
ESL = E // NCORES  # 128: per-core E slice (2 heads)
C = 128            # attention chunk
NCH = L // C       # 16 chunks per sequence
ROWS = N * L       # 8192 token rows, n-major
KO = E // 128      # 8 contraction chunks
RT = 1024          # projection row-tile (== L // 2)

F32 = mybir.dt.float32
BF16 = mybir.dt.bfloat16
AF = mybir.ActivationFunctionType
ALU = mybir.AluOpType


def _bc(ap, count, axis_pos=1):
    """Broadcast an AP by inserting a 0-stride dim of `count` at axis_pos."""
    new = list(ap.ap)
    new.insert(axis_pos, [0, count])
    return bass.AP(tensor=ap.tensor, offset=ap.offset, ap=new)


def build_nc() -> bass.Bass:
    nc = bass.Bass()

    # ---- I/O ----
    xqT = nc.declare_dram_parameter("xqT", [E, ROWS], BF16, isOutput=False)
    xkT = nc.declare_dram_parameter("xkT", [E, ROWS], BF16, isOutput=False)
    wqT = nc.declare_dram_parameter("wqT", [E, ESL], BF16, isOutput=False)
    wkT = nc.declare_dram_parameter("wkT", [E, ESL], BF16, isOutput=False)
    bq = nc.declare_dram_parameter("bq", [ESL], F32, isOutput=False)
    bk = nc.declare_dram_parameter("bk", [ESL], F32, isOutput=False)
    memsc = nc.declare_dram_parameter("memsc", [L, ESL], BF16, isOutput=False)
    lnw = nc.declare_dram_parameter("lnw", [ESL], F32, isOutput=False)
    lnb = nc.declare_dram_parameter("lnb", [ESL], F32, isOutput=False)
    out = nc.declare_dram_parameter("out", [L, N, ESL], F32, isOutput=True)

    cc_in = [nc.dram_tensor(f"cc_in{i}", [128, NCH, 2], F32) for i in range(N)]
    cc_out = [
        nc.dram_tensor(f"cc_out{i}", [128, NCH, 2], F32, addr_space="Shared")
        for i in range(N)
    ]

    with tile.TileContext(nc) as tc:
        _emit(nc, tc, xqT, xkT, wqT, wkT, bq, bk, memsc, lnw, lnb, out, cc_in, cc_out)
    return nc


def _emit(nc, tc, xqT, xkT, wqT, wkT, bq, bk, memsc, lnw, lnb, out, cc_in, cc_out):
    import contextlib

    ctx = contextlib.ExitStack()
    with ctx:
        singles = ctx.enter_context(tc.tile_pool(name="singles", bufs=1))

        # ---- constants ----
        ident = singles.tile([128, 128], BF16)
        make_identity(nc, ident)
        cmask = singles.tile([128, 128], F32)  # 1 where s(part) <= l(free)
        make_upper_triangular(nc, cmask, val=1.0, diag=True)
        cmask2 = singles.tile([128, 2, 128], F32)  # duplicated per head
        nc.vector.tensor_copy(cmask2[:, 0, :], cmask)
        nc.vector.tensor_copy(cmask2[:, 1, :], cmask)

        bq_sb = singles.tile([128, 1], F32)
        nc.sync.dma_start(out=bq_sb, in_=bq.rearrange("(p one) -> p one", one=1))
        bk_sb = singles.tile([128, 1], F32)
        nc.sync.dma_start(out=bk_sb, in_=bk.rearrange("(p one) -> p one", one=1))

        lnw_sb = singles.tile([128, ESL], F32)
        nc.sync.dma_start(out=lnw_sb, in_=_bc(lnw[:], 128, 0))
        lnb_sb = singles.tile([128, ESL], F32)
        nc.sync.dma_start(out=lnb_sb, in_=_bc(lnb[:], 128, 0))
        eps_sb = singles.tile([128, 1], F32)
        nc.vector.memset(eps_sb, LN_EPS)

        # lam*memory in [s%C, (chunk, e)] layout
        mem_sb = singles.tile([128, NCH, ESL], BF16)
        nc.sync.dma_start(out=mem_sb, in_=memsc.rearrange("(c p) e -> p c e", p=128))

        # ---- weights ----
        wq_sb = singles.tile([128, KO, ESL], BF16)
        nc.sync.dma_start(out=wq_sb, in_=wqT.rearrange("(ko ki) m -> ki ko m", ki=128))
        wk_sb = singles.tile([128, KO, ESL], BF16)
        nc.sync.dma_start(out=wk_sb, in_=wkT.rearrange("(ko ki) m -> ki ko m", ki=128))

        # ---- persistent activations; one tile per projection row-tile so
        # dependency ranges stay precise and attention can start early ----
        qT_t = [singles.tile([128, RT], BF16, name=f"qT{i}") for i in range(ROWS // RT)]
        kT_t = [singles.tile([128, RT], BF16, name=f"kT{i}") for i in range(ROWS // RT)]
        out_sb = singles.tile([128, N * NCH, ESL], F32)  # [l%C, (n,ch), e]
        # linear-attention state per batch: [(h,d), n, e_local(64)]
        a32 = singles.tile([128, N, D], F32)
        abf = singles.tile([128, N, D], BF16)

        xpool = ctx.enter_context(tc.tile_pool(name="xpool", bufs=2))
        ppool = ctx.enter_context(tc.tile_pool(name="ppool", bufs=2, space="PSUM"))
        tpool = ctx.enter_context(tc.tile_pool(name="tpool", bufs=2, space="PSUM"))
        spool = ctx.enter_context(tc.tile_pool(name="spool", bufs=2, space="PSUM"))
        opool = ctx.enter_context(tc.tile_pool(name="opool", bufs=2, space="PSUM"))
        apool = ctx.enter_context(tc.tile_pool(name="apool", bufs=3))
        stpool = ctx.enter_context(tc.tile_pool(name="stpool", bufs=2))

        def proj(rt):
            r0 = rt * RT
            for xdram, w_sb, bias_sb, dst in (
                (xqT, wq_sb, bq_sb, qT_t[rt]),
                (xkT, wk_sb, bk_sb, kT_t[rt]),
            ):
                xt = xpool.tile([128, KO, RT], BF16, tag="xt", name="xt")
                nc.sync.dma_start(
                    out=xt,
                    in_=xdram.rearrange("(ko ki) r -> ki ko r", ki=128)[
                        :, :, r0 : r0 + RT
                    ],
                )
                for st in range(RT // 512):
                    ps = ppool.tile([128, 512], F32, tag="ps", name="ps")
                    for ko in range(KO):
                        nc.tensor.matmul(
                            ps,
                            w_sb[:, ko, :],
                            xt[:, ko, st * 512 : (st + 1) * 512],
                            start=(ko == 0),
                            stop=(ko == KO - 1),
                        )
                    nc.scalar.activation(
                        out=dst[:, st * 512 : (st + 1) * 512],
                        in_=ps,
                        func=AF.Gelu,
                        bias=bias_sb,
                        scale=1.0,
                    )

        def attn(ni, ch):
            rt = 2 * ni + ch // 8
            c0 = (ch % 8) * C
            slot = ni * NCH + ch
            qTs = qT_t[rt][:, c0 : c0 + C]
            kTs = kT_t[rt][:, c0 : c0 + C]

            qnat_ps = tpool.tile([128, 128], BF16, tag="tp", name="qnat_ps")
            nc.tensor.transpose(qnat_ps, qTs, ident)
            knat_ps = tpool.tile([128, 128], BF16, tag="tp", name="knat_ps")
            nc.tensor.transpose(knat_ps, kTs, ident)

            mem_nat = apool.tile([128, 128], BF16, tag="mn", name="mem_nat")
            nc.vector.scalar_tensor_tensor(
                out=mem_nat,
                in0=qnat_ps,
                scalar=1.0 - LAM,
                in1=mem_sb[:, ch, :],
                op0=ALU.mult,
                op1=ALU.add,
            )
            knat = apool.tile([128, 128], BF16, tag="kn", name="knat")
            nc.scalar.copy(knat, knat_ps)

            st_sbufs = []
            for h in range(2):
                hs = slice(h * D, (h + 1) * D)
                st_ps = spool.tile([128, 128], F32, tag="st", name="st_ps")
                nc.tensor.matmul(st_ps, kTs[hs, :], qTs[hs, :], start=True, stop=True)
                st_sb = apool.tile([128, 128], BF16, tag="stsb", name="st_sb")
                nc.vector.scalar_tensor_tensor(
                    out=st_sb,
                    in0=st_ps,
                    scalar=1.0,
                    in1=cmask,
                    op0=ALU.mult,
                    op1=ALU.mult,
                )
                st_sbufs.append(st_sb)

            op_ps = opool.tile([128, 192], F32, tag="op", name="op_ps")
            for h in range(2):
                hs = slice(h * D, (h + 1) * D)
                nc.tensor.matmul(
                    op_ps[:, h * D : (h + 1) * D],
                    st_sbufs[h],
                    mem_nat[:, hs],
                    start=True,
                    stop=(ch == 0),
                )
                if ch > 0:
                    nc.tensor.matmul(
                        op_ps[:, h * D : (h + 1) * D],
                        qTs[hs, :],
                        abf[hs, ni, :],
                        start=False,
                        stop=True,
                    )
            i_d = None
            for h in range(2):
                hs = slice(h * D, (h + 1) * D)
                i_d = nc.tensor.matmul(
                    op_ps[hs, 128:192],
                    knat[:, hs],
                    mem_nat[:, hs],
                    start=True,
                    stop=True,
                )

            i_ev = nc.scalar.copy(out_sb[:, slot, :], op_ps[:, 0:128])
            # same PSUM bank: don't read cols 0:128 while PE writes 128:192
            tile.add_dep_helper(i_ev.ins, i_d.ins, reason="op_ps bank serialize")
            if ch == 0:
                nc.vector.tensor_copy(a32[:, ni, :], op_ps[:, 128:192])
            else:
                nc.vector.scalar_tensor_tensor(
                    out=a32[:, ni, :],
                    in0=op_ps[:, 128:192],
                    scalar=1.0,
                    in1=a32[:, ni, :],
                    op0=ALU.mult,
                    op1=ALU.add,
                )
            nc.scalar.copy(abf[:, ni, :], a32[:, ni, :])

        def stats_and_ar(ni):
            # batched per-chunk sums: mean/8 = sum(x)/1024, E[x^2]/8 = sum(x^2)/1024
            slab = out_sb[:, ni * NCH : (ni + 1) * NCH, :]
            sq = stpool.tile([128, NCH, ESL], F32, tag="sq", name="sq")
            nc.gpsimd.tensor_mul(sq, slab, slab)
            sums = stpool.tile([128, NCH, 2], F32, tag="sums", name="sums")
            nc.vector.tensor_reduce(
                sums[:, :, 0], slab, axis=mybir.AxisListType.X, op=ALU.add
            )
            nc.vector.tensor_reduce(
                sums[:, :, 1], sq, axis=mybir.AxisListType.X, op=ALU.add
            )
            stats = stpool.tile([128, NCH, 2], F32, tag="stats", name="stats")
            nc.scalar.mul(stats, sums, 1.0 / (ESL * NCORES))
            nc.sync.dma_start(out=cc_in[ni][:, :, :], in_=stats)
            nc.gpsimd.collective_compute(
                "AllReduce",
                ALU.add,
                replica_groups=[list(range(NCORES))],
                ins=[cc_in[ni][:, :, :]],
                outs=[cc_out[ni][:, :, :]],
            )

        def ln_final(ni):
            g = stpool.tile([128, NCH, 2], F32, tag="g", name="g")
            nc.sync.dma_start(out=g, in_=cc_out[ni][:, :, :])
            mu = g[:, :, 0]
            musq = stpool.tile([128, NCH], F32, tag="musq", name="musq")
            nc.vector.tensor_mul(musq, mu, mu)
            var = stpool.tile([128, NCH], F32, tag="var", name="var")
            nc.vector.tensor_sub(var, g[:, :, 1], musq)
            rstd = stpool.tile([128, NCH], F32, tag="rstd", name="rstd")
            nc.scalar.activation(out=rstd, in_=var, func=AF.Sqrt, bias=eps_sb, scale=1.0)
            nc.vector.reciprocal(rstd, rstd)
            nmr = stpool.tile([128, NCH], F32, tag="nmr", name="nmr")
            nc.vector.scalar_tensor_tensor(
                out=nmr, in0=mu, scalar=-1.0, op0=ALU.mult, in1=rstd, op1=ALU.mult
            )
            # (x - mu) * rstd  per chunk (per-partition scalars), in place
            for ch in range(NCH):
                slot = ni * NCH + ch
                nc.scalar.activation(
                    out=out_sb[:, slot, :],
                    in_=out_sb[:, slot, :],
                    func=AF.Identity,
                    bias=nmr[:, ch : ch + 1],
                    scale=rstd[:, ch : ch + 1],
                )
            # * ln_w + ln_b on the idle GPSIMD engine, batched
            for ch in range(NCH):
                slot = ni * NCH + ch
                nc.gpsimd.tensor_mul(out_sb[:, slot, :], out_sb[:, slot, :], lnw_sb)
                nc.gpsimd.tensor_add(out_sb[:, slot, :], out_sb[:, slot, :], lnb_sb)
            sl = out_sb[:, ni * NCH : (ni + 1) * NCH, :]
            nc.sync.dma_start(
                out=out[:, ni, :].rearrange("(c p) e -> p c e", p=128),
                in_=sl,
            )

        # ---- interleaved schedule ----
        for ni in range(N):
            for half in range(2):
                proj(2 * ni + half)
                for ch in range(8 * half, 8 * half + 8):
                    attn(ni, ch)
            stats_and_ar(ni)
            if ni > 0:
                ln_final(ni - 1)
        ln_final(N - 1)


_NC_CACHE = None


def _get_nc():
    global _NC_CACHE
    if _NC_CACHE is None:
        _install_patch()
        _NC_CACHE = build_nc()
    return _NC_CACHE


def kernel(**inputs) -> np.ndarray:
    query = np.asarray(inputs["query"], np.float32)  # (L, N, E)
    key = np.asarray(inputs["key"], np.float32)
    Wq = np.asarray(inputs["Wq"], np.float32)        # (E, E)
    bq = np.asarray(inputs["bq"], np.float32)
    Wk = np.asarray(inputs["Wk"], np.float32)
    bk = np.asarray(inputs["bk"], np.float32)
    memory = np.asarray(inputs["memory"], np.float32)  # (MAXL, E)
    ln_w = np.asarray(inputs["ln_w"], np.float32)
    ln_b = np.asarray(inputs["ln_b"], np.float32)

    bf = ml_dtypes.bfloat16
    # token rows n-major: row = n*L + l ; transposed to [E, ROWS]
    xqT = np.ascontiguousarray(query.transpose(2, 1, 0).reshape(E, ROWS)).astype(bf)
    xkT = np.ascontiguousarray(key.transpose(2, 1, 0).reshape(E, ROWS)).astype(bf)

    nc = _get_nc()
    in_maps = []
    for c in range(NCORES):
        sl = slice(c * ESL, (c + 1) * ESL)
        in_maps.append(
            {
                "xqT": xqT,
                "xkT": xkT,
                "wqT": np.ascontiguousarray(Wq[sl, :].T).astype(bf),
                "wkT": np.ascontiguousarray(Wk[sl, :].T).astype(bf),
                "bq": np.ascontiguousarray(bq[sl]),
                "bk": np.ascontiguousarray(bk[sl]),
                "memsc": (LAM * memory[:L, sl]).astype(bf),
                "lnw": np.ascontiguousarray(ln_w[sl]),
                "lnb": np.ascontiguousarray(ln_b[sl]),
            }
        )

    res = run_bass_kernel_spmd(nc, in_maps, core_ids=list(range(NCORES)))
    return np.concatenate([res.results[c]["out"] for c in range(NCORES)], axis=2)
